# revision 63
# baseline (speedup 1.0000x reference)
"""MetaUpScale (scale=2) Trainium2 Bass kernel.

Math: for output pixel (i, j) = (2y+a, 2x+b), the reference computes
    out[i, j, o] = sum_{p,q,c} padded_feature[y+p-1+a0.., ...]  -- precisely:
    i' = floor(i/2) = y, window rows y+p-1, cols x+q-1 (zero padded),
    weights w = MLP(v_i, v_j, 1/2) where v_i = 0.5*(i%2), v_j = 0.5*(j%2).
With scale=2 the MLP input only takes 4 distinct values (parities a, b), so
the per-pixel MLP collapses to 4 weight sets computed on the host, and the
device op is 4 interleaved 3x3 convolutions done as 9 accumulating PE
matmuls (contract C=64, M=12=(a,b,o), N=512=(4 rows x 128 cols)) per batch.

Sharding: 16 low-res rows per core (x8 cores), halo rows come in via
host-prepared zero-padded per-core slices [64, 18, 130].
"""

import numpy as np

import concourse.bacc as bacc
import concourse.bass as bass
import concourse.mybir as mybir
import concourse.tile as tile
from concourse.bass_utils import run_bass_kernel_spmd

H, W, C = 128, 128, 64
K = 3
OUT_C = 3
SCALE = 2
N_CORES = 8
ROWS_PER_CORE = H // N_CORES          # 16 low-res rows
HALO_ROWS = ROWS_PER_CORE + 2         # 18 with halo
WPAD = W + 2                          # 130, zero column padding
M_OUT = SCALE * SCALE * OUT_C         # 12 output channels (a, b, o)
ROW_BATCH = 4                         # rows per PSUM batch -> N = 4*128 = 512
N_BATCHES = ROWS_PER_CORE // ROW_BATCH

_CACHE = {}


WT_W = K * K * M_OUT                        # 108 weight columns (first)
CHUNK_R = ROW_BATCH + K - 1                 # 6 halo rows per batch chunk
CHUNK_W = CHUNK_R * WPAD                    # 910
FUSED_W = WT_W + N_BATCHES * CHUNK_W        # wt + 4 overlapping chunks
# PE tap packing mode:
#   "none" - 9 sequential matmuls per batch
#   "row"  - duplicate operands in partitions 64..127, alternate 64-row PE
#            groups (concurrent accumulation into the same psum region)
#   "col"  - alternate 32-col PE groups writing disjoint psum partition
#            ranges (0-11 / 32-43), merged by the psum->sbuf add
TAP_MODE = "none"
TWO_TAP = TAP_MODE == "row"
COL_OFS = 64  # fp32 matmuls span two col-groups; dst must be 64-aligned
PSUM_BUFS = 4
# dep-free dummy matmuls issued while the input DMA is in flight: they keep
# the PE busy through the HAM activity window so the real burst runs warm.
# Plain f32 (4 cycles/row) so each N=512 dummy burns ~0.9-1.7us of PE time.
N_WARM = 0


def _build_program(mm_dtype):
    # Bacc (not raw Bass): its compile() splits sync waits so instructions
    # respect the 1-wait hardware limit walrus enforces.
    #
    # K-pair packing: tap (p=0,q) weights/features occupy contraction rows
    # 0-63 and tap (p=1,q) rows 64-127 of a single K=128 matmul. Rows 64-127
    # of each chunk tile hold the SAME feature shifted one halo row (second
    # DMA from DRAM), so one AP offset serves both taps. Taps (p=2,q) run
    # solo at K=64. 6 matmuls per batch instead of 9.
    nc = bacc.Bacc("TRN2", target_bir_lowering=False, debug=False)
    f32 = mybir.dt.float32
    fused_in = nc.dram_tensor("fused", [C, FUSED_W], mm_dtype, kind="ExternalInput")
    # weights: cols 0-35 = pairs q0..q2 (rows 0-63 p=0, rows 64-127 p=1),
    # cols 36-71 = solos p=2 (rows 0-63)
    wtp_in = nc.dram_tensor("wtp", [2 * C, 6 * M_OUT], mm_dtype, kind="ExternalInput")
    out_d = nc.dram_tensor(
        "out", [M_OUT, ROWS_PER_CORE, W], f32, kind="ExternalOutput"
    )
    # pre-shifted copy for the paired taps: they read rows 0..3 only, so
    # transfer exactly ROW_BATCH rows (chunk rows 1..4)
    DUP_W = ROW_BATCH * WPAD

    with tile.TileContext(nc) as tc:
        with (
            tc.tile_pool(name="sbuf", bufs=1) as pool,
            tc.tile_pool(
                name="psum", bufs=PSUM_BUFS, space=bass.MemorySpace.PSUM
            ) as psum,
        ):
            out_s = pool.tile([M_OUT, ROWS_PER_CORE, W], f32)
            wtp = pool.tile([2 * C, 6 * M_OUT], mm_dtype)
            nc.scalar.dma_start(wtp[:], wtp_in[:])

            chunks = []
            for t in range(N_BATCHES):
                ck = pool.tile([2 * C, CHUNK_W], mm_dtype, tag=f"chunk{t}")
                lo = WT_W + t * CHUNK_W
                # rows 0-63: the chunk; rows 64-127: same, shifted one row.
                # Two HWDGE queues (SP / Activation) so both transfer at once.
                # (Splitting further onto SWDGE queues was measured SLOWER —
                # SWDGE first-byte latency exceeds the parallelism gain.)
                nc.sync.dma_start(ck[:C], fused_in[:, lo : lo + CHUNK_W])
                nc.scalar.dma_start(
                    ck[C:, :DUP_W], fused_in[:, lo + WPAD : lo + WPAD + DUP_W]
                )
                chunks.append(ck)

            for t in range(N_BATCHES):
                # flat [12, 512] psum view: fp32r matmuls only hit the
                # 1-cycle/row fast path when the dst innermost run is >=256
                ps = psum.tile([M_OUT, ROW_BATCH * W], f32)
                f3_pair = chunks[t][:].rearrange("c (r w) -> c r w", w=WPAD)
                f3_solo = chunks[t][:C].rearrange("c (r w) -> c r w", w=WPAD)
                # output rows y = 4t + r need feature row u = y + p - 1 =
                # chunk-local row p + r; output col x needs padded col q + x.
                for idx in range(2 * K):
                    q = idx % K
                    if idx < K:  # paired taps (0,q) + (1,q), K = 128
                        lhsT = wtp[:, q * M_OUT : (q + 1) * M_OUT]
                        rhs = f3_pair[:, 0:ROW_BATCH, q : q + W]
                    else:  # solo tap (2,q), K = 64
                        lhsT = wtp[:C, (K + q) * M_OUT : (K + q + 1) * M_OUT]
                        rhs = f3_solo[:, 2 : 2 + ROW_BATCH, q : q + W]
                    nc.tensor.matmul(
                        ps[:],
                        lhsT,
                        rhs,
                        start=(idx == 0),
                        stop=(idx == 2 * K - 1),
                    )
                nc.vector.tensor_copy(
                    out_s[:, 4 * t : 4 * t + ROW_BATCH],
                    ps[:].rearrange("m (r w) -> m r w", w=W),
                )
                eng = nc.scalar if t % 2 == 0 else nc.sync
                eng.dma_start(
                    out_d[:, 4 * t : 4 * t + ROW_BATCH],
                    out_s[:, 4 * t : 4 * t + ROW_BATCH],
                )

    nc.finalize()
    return nc


def _get_program(mm_dtype_name="float32r"):
    key = (mm_dtype_name, N_WARM, PSUM_BUFS)
    if key not in _CACHE:
        _CACHE[key] = _build_program(getattr(mybir.dt, mm_dtype_name))
    return _CACHE[key]


def _host_weights(kernel_1, bias_1, kernel_2, bias_2, kernel_3, bias_3):
    """4 parity rows through the MLP -> wt [C, 9, 12] fp32."""
    v4 = np.array(
        [[0.5 * a, 0.5 * b, 0.5] for a in range(2) for b in range(2)],
        dtype=np.float32,
    )
    h = np.maximum(v4 @ kernel_1 + bias_1, 0.0).astype(np.float32)
    h = np.maximum(h @ kernel_2 + bias_2, 0.0).astype(np.float32)
    w = (h @ kernel_3 + bias_3).astype(np.float32)          # [4, 3*3*C*3]
    w = w.reshape(2, 2, K, K, C, OUT_C)                     # [a, b, p, q, c, o]
    wt = np.transpose(w, (4, 2, 3, 0, 1, 5))                # [c, p, q, a, b, o]
    return np.ascontiguousarray(wt.reshape(C, K * K, M_OUT), dtype=np.float32)


def _host_feat_slices(feature):
    """Per-core zero-padded [C, 19, 130] slices of feature^T (one trailing
    zero row so the per-batch 7-row chunks slice uniformly)."""
    featT = np.ascontiguousarray(np.transpose(feature, (2, 0, 1)))  # [C, H, W]
    slices = []
    for k in range(N_CORES):
        fs = np.zeros((C, HALO_ROWS + 1, WPAD), dtype=np.float32)
        y0 = k * ROWS_PER_CORE
        lo = max(y0 - 1, 0)
        hi = min(y0 + ROWS_PER_CORE + 1, H)
        fs[:, (lo - (y0 - 1)) : (hi - (y0 - 1)), 1 : 1 + W] = featT[:, lo:hi, :]
        slices.append(fs)
    return slices


def _host_pack(wt, fslice):
    """wt [C, 9, 12] + one core's [C, 19, 130] slice -> fused [C, FUSED_W]."""
    parts = [wt.reshape(C, WT_W)] + [
        fslice[:, ROW_BATCH * t : ROW_BATCH * t + CHUNK_R].reshape(C, -1)
        for t in range(N_BATCHES)
    ]
    return np.ascontiguousarray(np.concatenate(parts, axis=1))


def _host_pack_wtp(wt):
    """wt [C, 9(p*3+q), 12] -> [2C, 72]: cols 0-35 pair q0..2 (p=0 top /
    p=1 bottom), cols 36-71 solo p=2 (top, zero bottom)."""
    w3 = wt.reshape(C, K, K, M_OUT)                      # [c, p, q, m]
    pairs = np.concatenate(
        [w3[:, 0].reshape(C, K * M_OUT), w3[:, 1].reshape(C, K * M_OUT)], axis=0
    )                                                    # [2C, 36]
    solos = np.concatenate(
        [w3[:, 2].reshape(C, K * M_OUT), np.zeros((C, K * M_OUT), np.float32)],
        axis=0,
    )                                                    # [2C, 36]
    return np.ascontiguousarray(np.concatenate([pairs, solos], axis=1))


def _assemble(results):
    """Per-core [m_rows, 16, 128] outputs -> full [H*W*4, 3]."""
    big = np.empty((SCALE * H, SCALE * W, OUT_C), dtype=np.float32)
    for k in range(N_CORES):
        arr = np.asarray(results[k]["out"])
        if arr.shape[0] > M_OUT:  # col-tiled: sum the two chain slices
            ofs = arr.shape[0] - M_OUT
            arr = arr[0:M_OUT] + arr[ofs : ofs + M_OUT]
        o = arr.reshape(2, 2, OUT_C, ROWS_PER_CORE, W)
        # [a, b, o, y, x] -> [y, a, x, b, o]
        t = np.transpose(o, (3, 0, 4, 1, 2)).reshape(
            SCALE * ROWS_PER_CORE, SCALE * W, OUT_C
        )
        big[SCALE * ROWS_PER_CORE * k : SCALE * ROWS_PER_CORE * (k + 1)] = t
    return big.reshape(-1, OUT_C)


def run_device(inputs, mm_dtype_name="float32r", trace=False, **run_kwargs):
    feature = np.asarray(inputs["feature"], dtype=np.float32)
    scale = int(np.asarray(inputs["scale"]))
    assert scale == SCALE, f"kernel specialized for scale=2, got {scale}"
    assert feature.shape == (H, W, C)

    wt = _host_weights(
        np.asarray(inputs["kernel_1"], np.float32),
        np.asarray(inputs["bias_1"], np.float32),
        np.asarray(inputs["kernel_2"], np.float32),
        np.asarray(inputs["bias_2"], np.float32),
        np.asarray(inputs["kernel_3"], np.float32),
        np.asarray(inputs["bias_3"], np.float32),
    )
    slices = _host_feat_slices(feature)
    wtp = _host_pack_wtp(wt)
    in_maps = [
        {"fused": _host_pack(wt, slices[k]), "wtp": wtp} for k in range(N_CORES)
    ]
    nc = _get_program(mm_dtype_name)
    res = run_bass_kernel_spmd(
        nc, in_maps, list(range(N_CORES)), trace=trace, **run_kwargs
    )
    return _assemble(res.results), res


def kernel(**inputs) -> np.ndarray:
    out, _ = run_device(inputs)
    return out


# revision 64
# speedup vs baseline: 1.0294x; 1.0294x over previous
"""MetaUpScale (scale=2) Trainium2 Bass kernel.

Math: for output pixel (i, j) = (2y+a, 2x+b), the reference computes
    out[i, j, o] = sum_{p,q,c} padded_feature[y+p-1+a0.., ...]  -- precisely:
    i' = floor(i/2) = y, window rows y+p-1, cols x+q-1 (zero padded),
    weights w = MLP(v_i, v_j, 1/2) where v_i = 0.5*(i%2), v_j = 0.5*(j%2).
With scale=2 the MLP input only takes 4 distinct values (parities a, b), so
the per-pixel MLP collapses to 4 weight sets computed on the host, and the
device op is 4 interleaved 3x3 convolutions done as 9 accumulating PE
matmuls (contract C=64, M=12=(a,b,o), N=512=(4 rows x 128 cols)) per batch.

Sharding: 16 low-res rows per core (x8 cores), halo rows come in via
host-prepared zero-padded per-core slices [64, 18, 130].
"""

import numpy as np

import concourse.bacc as bacc
import concourse.bass as bass
import concourse.mybir as mybir
import concourse.tile as tile
from concourse.bass_utils import run_bass_kernel_spmd

H, W, C = 128, 128, 64
K = 3
OUT_C = 3
SCALE = 2
N_CORES = 8
ROWS_PER_CORE = H // N_CORES          # 16 low-res rows
HALO_ROWS = ROWS_PER_CORE + 2         # 18 with halo
WPAD = W + 2                          # 130, zero column padding
M_OUT = SCALE * SCALE * OUT_C         # 12 output channels (a, b, o)
ROW_BATCH = 4                         # rows per PSUM batch -> N = 4*128 = 512
N_BATCHES = ROWS_PER_CORE // ROW_BATCH

_CACHE = {}


WT_W = K * K * M_OUT                        # 108 weight columns (first)
CHUNK_R = ROW_BATCH + K - 1                 # 6 halo rows per batch chunk
CHUNK_W = CHUNK_R * WPAD                    # 910
FUSED_W = WT_W + N_BATCHES * CHUNK_W        # wt + 4 overlapping chunks
# PE tap packing mode:
#   "none" - 9 sequential matmuls per batch
#   "row"  - duplicate operands in partitions 64..127, alternate 64-row PE
#            groups (concurrent accumulation into the same psum region)
#   "col"  - alternate 32-col PE groups writing disjoint psum partition
#            ranges (0-11 / 32-43), merged by the psum->sbuf add
TAP_MODE = "none"
TWO_TAP = TAP_MODE == "row"
COL_OFS = 64  # fp32 matmuls span two col-groups; dst must be 64-aligned
PSUM_BUFS = 4
# dep-free dummy matmuls issued while the input DMA is in flight: they keep
# the PE busy through the HAM activity window so the real burst runs warm.
# Plain f32 (4 cycles/row) so each N=512 dummy burns ~0.9-1.7us of PE time.
N_WARM = 0


def _build_program(mm_dtype):
    # Bacc (not raw Bass): its compile() splits sync waits so instructions
    # respect the 1-wait hardware limit walrus enforces.
    #
    # K-pair packing: tap (p=0,q) weights/features occupy contraction rows
    # 0-63 and tap (p=1,q) rows 64-127 of a single K=128 matmul. Rows 64-127
    # of each chunk tile hold the SAME feature shifted one halo row (second
    # DMA from DRAM), so one AP offset serves both taps. Taps (p=2,q) run
    # solo at K=64. 6 matmuls per batch instead of 9.
    nc = bacc.Bacc("TRN2", target_bir_lowering=False, debug=False)
    f32 = mybir.dt.float32
    fused_in = nc.dram_tensor("fused", [C, FUSED_W], mm_dtype, kind="ExternalInput")
    # weights: cols 0-35 = pairs q0..q2 (rows 0-63 p=0, rows 64-127 p=1),
    # cols 36-71 = solos p=2 (rows 0-63)
    wtp_in = nc.dram_tensor("wtp", [2 * C, 6 * M_OUT], mm_dtype, kind="ExternalInput")
    out_d = nc.dram_tensor(
        "out", [M_OUT, ROWS_PER_CORE, W], f32, kind="ExternalOutput"
    )
    # pre-shifted copy for the paired taps: they read rows 0..3 only, so
    # transfer exactly ROW_BATCH rows (chunk rows 1..4)
    DUP_W = ROW_BATCH * WPAD

    with tile.TileContext(nc) as tc:
        with (
            tc.tile_pool(name="sbuf", bufs=1) as pool,
            tc.tile_pool(
                name="psum", bufs=PSUM_BUFS, space=bass.MemorySpace.PSUM
            ) as psum,
        ):
            out_s = pool.tile([M_OUT, ROWS_PER_CORE, W], f32)
            wtp = pool.tile([2 * C, 6 * M_OUT], mm_dtype)
            nc.scalar.dma_start(wtp[:], wtp_in[:])

            chunks = []
            for t in range(N_BATCHES):
                ck = pool.tile([2 * C, CHUNK_W], mm_dtype, tag=f"chunk{t}")
                lo = WT_W + t * CHUNK_W
                # rows 0-63: the chunk; rows 64-127: same, shifted one row.
                # Two HWDGE queues (SP / Activation) so both transfer at once.
                # (Splitting further onto SWDGE queues was measured SLOWER —
                # SWDGE first-byte latency exceeds the parallelism gain.)
                nc.sync.dma_start(ck[:C], fused_in[:, lo : lo + CHUNK_W])
                nc.scalar.dma_start(
                    ck[C:, :DUP_W], fused_in[:, lo + WPAD : lo + WPAD + DUP_W]
                )
                chunks.append(ck)

            for t in range(N_BATCHES):
                # flat [12, 512] psum view: fp32r matmuls only hit the
                # 1-cycle/row fast path when the dst innermost run is >=256
                ps = psum.tile([M_OUT, ROW_BATCH * W], f32)
                f3_pair = chunks[t][:].rearrange("c (r w) -> c r w", w=WPAD)
                f3_solo = chunks[t][:C].rearrange("c (r w) -> c r w", w=WPAD)
                # output rows y = 4t + r need feature row u = y + p - 1 =
                # chunk-local row p + r; output col x needs padded col q + x.
                for idx in range(2 * K):
                    q = idx % K
                    if idx < K:  # paired taps (0,q) + (1,q), K = 128
                        lhsT = wtp[:, q * M_OUT : (q + 1) * M_OUT]
                        rhs = f3_pair[:, 0:ROW_BATCH, q : q + W]
                    else:  # solo tap (2,q), K = 64
                        lhsT = wtp[:C, (K + q) * M_OUT : (K + q + 1) * M_OUT]
                        rhs = f3_solo[:, 2 : 2 + ROW_BATCH, q : q + W]
                    nc.tensor.matmul(
                        ps[:],
                        lhsT,
                        rhs,
                        start=(idx == 0),
                        stop=(idx == 2 * K - 1),
                    )
                ps3 = ps[:].rearrange("m (r w) -> m r w", w=W)
                if t == N_BATCHES - 1:
                    # tail: split the last copy+DMA so the first half's DMA
                    # overlaps the second half's copy
                    hb = ROW_BATCH // 2
                    for h in range(2):
                        r0 = 4 * t + h * hb
                        nc.vector.tensor_copy(
                            out_s[:, r0 : r0 + hb], ps3[:, h * hb : h * hb + hb]
                        )
                        eng = nc.scalar if h == 0 else nc.sync
                        eng.dma_start(
                            out_d[:, r0 : r0 + hb], out_s[:, r0 : r0 + hb]
                        )
                else:
                    nc.vector.tensor_copy(out_s[:, 4 * t : 4 * t + ROW_BATCH], ps3)
                    eng = nc.scalar if t % 2 == 0 else nc.sync
                    eng.dma_start(
                        out_d[:, 4 * t : 4 * t + ROW_BATCH],
                        out_s[:, 4 * t : 4 * t + ROW_BATCH],
                    )

    nc.finalize()
    return nc


def _get_program(mm_dtype_name="float32r"):
    key = (mm_dtype_name, N_WARM, PSUM_BUFS)
    if key not in _CACHE:
        _CACHE[key] = _build_program(getattr(mybir.dt, mm_dtype_name))
    return _CACHE[key]


def _host_weights(kernel_1, bias_1, kernel_2, bias_2, kernel_3, bias_3):
    """4 parity rows through the MLP -> wt [C, 9, 12] fp32."""
    v4 = np.array(
        [[0.5 * a, 0.5 * b, 0.5] for a in range(2) for b in range(2)],
        dtype=np.float32,
    )
    h = np.maximum(v4 @ kernel_1 + bias_1, 0.0).astype(np.float32)
    h = np.maximum(h @ kernel_2 + bias_2, 0.0).astype(np.float32)
    w = (h @ kernel_3 + bias_3).astype(np.float32)          # [4, 3*3*C*3]
    w = w.reshape(2, 2, K, K, C, OUT_C)                     # [a, b, p, q, c, o]
    wt = np.transpose(w, (4, 2, 3, 0, 1, 5))                # [c, p, q, a, b, o]
    return np.ascontiguousarray(wt.reshape(C, K * K, M_OUT), dtype=np.float32)


def _host_feat_slices(feature):
    """Per-core zero-padded [C, 19, 130] slices of feature^T (one trailing
    zero row so the per-batch 7-row chunks slice uniformly)."""
    featT = np.ascontiguousarray(np.transpose(feature, (2, 0, 1)))  # [C, H, W]
    slices = []
    for k in range(N_CORES):
        fs = np.zeros((C, HALO_ROWS + 1, WPAD), dtype=np.float32)
        y0 = k * ROWS_PER_CORE
        lo = max(y0 - 1, 0)
        hi = min(y0 + ROWS_PER_CORE + 1, H)
        fs[:, (lo - (y0 - 1)) : (hi - (y0 - 1)), 1 : 1 + W] = featT[:, lo:hi, :]
        slices.append(fs)
    return slices


def _host_pack(wt, fslice):
    """wt [C, 9, 12] + one core's [C, 19, 130] slice -> fused [C, FUSED_W]."""
    parts = [wt.reshape(C, WT_W)] + [
        fslice[:, ROW_BATCH * t : ROW_BATCH * t + CHUNK_R].reshape(C, -1)
        for t in range(N_BATCHES)
    ]
    return np.ascontiguousarray(np.concatenate(parts, axis=1))


def _host_pack_wtp(wt):
    """wt [C, 9(p*3+q), 12] -> [2C, 72]: cols 0-35 pair q0..2 (p=0 top /
    p=1 bottom), cols 36-71 solo p=2 (top, zero bottom)."""
    w3 = wt.reshape(C, K, K, M_OUT)                      # [c, p, q, m]
    pairs = np.concatenate(
        [w3[:, 0].reshape(C, K * M_OUT), w3[:, 1].reshape(C, K * M_OUT)], axis=0
    )                                                    # [2C, 36]
    solos = np.concatenate(
        [w3[:, 2].reshape(C, K * M_OUT), np.zeros((C, K * M_OUT), np.float32)],
        axis=0,
    )                                                    # [2C, 36]
    return np.ascontiguousarray(np.concatenate([pairs, solos], axis=1))


def _assemble(results):
    """Per-core [m_rows, 16, 128] outputs -> full [H*W*4, 3]."""
    big = np.empty((SCALE * H, SCALE * W, OUT_C), dtype=np.float32)
    for k in range(N_CORES):
        arr = np.asarray(results[k]["out"])
        if arr.shape[0] > M_OUT:  # col-tiled: sum the two chain slices
            ofs = arr.shape[0] - M_OUT
            arr = arr[0:M_OUT] + arr[ofs : ofs + M_OUT]
        o = arr.reshape(2, 2, OUT_C, ROWS_PER_CORE, W)
        # [a, b, o, y, x] -> [y, a, x, b, o]
        t = np.transpose(o, (3, 0, 4, 1, 2)).reshape(
            SCALE * ROWS_PER_CORE, SCALE * W, OUT_C
        )
        big[SCALE * ROWS_PER_CORE * k : SCALE * ROWS_PER_CORE * (k + 1)] = t
    return big.reshape(-1, OUT_C)


def run_device(inputs, mm_dtype_name="float32r", trace=False, **run_kwargs):
    feature = np.asarray(inputs["feature"], dtype=np.float32)
    scale = int(np.asarray(inputs["scale"]))
    assert scale == SCALE, f"kernel specialized for scale=2, got {scale}"
    assert feature.shape == (H, W, C)

    wt = _host_weights(
        np.asarray(inputs["kernel_1"], np.float32),
        np.asarray(inputs["bias_1"], np.float32),
        np.asarray(inputs["kernel_2"], np.float32),
        np.asarray(inputs["bias_2"], np.float32),
        np.asarray(inputs["kernel_3"], np.float32),
        np.asarray(inputs["bias_3"], np.float32),
    )
    slices = _host_feat_slices(feature)
    wtp = _host_pack_wtp(wt)
    in_maps = [
        {"fused": _host_pack(wt, slices[k]), "wtp": wtp} for k in range(N_CORES)
    ]
    nc = _get_program(mm_dtype_name)
    res = run_bass_kernel_spmd(
        nc, in_maps, list(range(N_CORES)), trace=trace, **run_kwargs
    )
    return _assemble(res.results), res


def kernel(**inputs) -> np.ndarray:
    out, _ = run_device(inputs)
    return out


# revision 65
# speedup vs baseline: 1.0967x; 1.0654x over previous
"""MetaUpScale (scale=2) Trainium2 Bass kernel.

Math: for output pixel (i, j) = (2y+a, 2x+b), the reference computes
    out[i, j, o] = sum_{p,q,c} padded_feature[y+p-1+a0.., ...]  -- precisely:
    i' = floor(i/2) = y, window rows y+p-1, cols x+q-1 (zero padded),
    weights w = MLP(v_i, v_j, 1/2) where v_i = 0.5*(i%2), v_j = 0.5*(j%2).
With scale=2 the MLP input only takes 4 distinct values (parities a, b), so
the per-pixel MLP collapses to 4 weight sets computed on the host, and the
device op is 4 interleaved 3x3 convolutions done as 9 accumulating PE
matmuls (contract C=64, M=12=(a,b,o), N=512=(4 rows x 128 cols)) per batch.

Sharding: 16 low-res rows per core (x8 cores), halo rows come in via
host-prepared zero-padded per-core slices [64, 18, 130].
"""

import numpy as np

import concourse.bacc as bacc
import concourse.bass as bass
import concourse.mybir as mybir
import concourse.tile as tile
from concourse.bass_utils import run_bass_kernel_spmd

H, W, C = 128, 128, 64
K = 3
OUT_C = 3
SCALE = 2
N_CORES = 8
ROWS_PER_CORE = H // N_CORES          # 16 low-res rows
HALO_ROWS = ROWS_PER_CORE + 2         # 18 with halo
WPAD = W + 2                          # 130, zero column padding
M_OUT = SCALE * SCALE * OUT_C         # 12 output channels (a, b, o)
ROW_BATCH = 4                         # rows per PSUM batch -> N = 4*128 = 512
N_BATCHES = ROWS_PER_CORE // ROW_BATCH

_CACHE = {}


WT_W = K * K * M_OUT                        # 108 weight columns (first)
CHUNK_R = ROW_BATCH + K - 1                 # 6 halo rows per batch chunk
CHUNK_W = CHUNK_R * WPAD                    # 910
FUSED_W = WT_W + N_BATCHES * CHUNK_W        # wt + 4 overlapping chunks
# PE tap packing mode:
#   "none" - 9 sequential matmuls per batch
#   "row"  - duplicate operands in partitions 64..127, alternate 64-row PE
#            groups (concurrent accumulation into the same psum region)
#   "col"  - alternate 32-col PE groups writing disjoint psum partition
#            ranges (0-11 / 32-43), merged by the psum->sbuf add
TAP_MODE = "none"
TWO_TAP = TAP_MODE == "row"
COL_OFS = 64  # fp32 matmuls span two col-groups; dst must be 64-aligned
PSUM_BUFS = 4
# dep-free dummy matmuls issued while the input DMA is in flight: they keep
# the PE busy through the HAM activity window so the real burst runs warm.
# Plain f32 (4 cycles/row) so each N=512 dummy burns ~0.9-1.7us of PE time.
N_WARM = 0


def _build_program(mm_dtype):
    # Bacc (not raw Bass): its compile() splits sync waits so instructions
    # respect the 1-wait hardware limit walrus enforces.
    #
    # K-pair packing: tap (p=0,q) weights/features occupy contraction rows
    # 0-63 and tap (p=1,q) rows 64-127 of a single K=128 matmul. Rows 64-127
    # of each chunk tile hold the SAME feature shifted one halo row (second
    # DMA from DRAM), so one AP offset serves both taps. Taps (p=2,q) run
    # solo at K=64. 6 matmuls per batch instead of 9.
    nc = bacc.Bacc("TRN2", target_bir_lowering=False, debug=False)
    f32 = mybir.dt.float32
    fused_in = nc.dram_tensor("fused", [C, FUSED_W], mm_dtype, kind="ExternalInput")
    # weights: cols 0-35 = pairs q0..q2 (rows 0-63 p=0, rows 64-127 p=1),
    # cols 36-71 = solos p=2 (rows 0-63)
    wtp_in = nc.dram_tensor("wtp", [2 * C, 6 * M_OUT], mm_dtype, kind="ExternalInput")
    out_d = nc.dram_tensor(
        "out", [M_OUT, ROWS_PER_CORE, W], f32, kind="ExternalOutput"
    )
    # pre-shifted copy for the paired taps: they read rows 0..3 only, so
    # transfer exactly ROW_BATCH rows (chunk rows 1..4)
    DUP_W = ROW_BATCH * WPAD

    with tile.TileContext(nc) as tc:
        with (
            tc.tile_pool(name="sbuf", bufs=1) as pool,
            tc.tile_pool(
                name="psum", bufs=PSUM_BUFS, space=bass.MemorySpace.PSUM
            ) as psum,
        ):
            out_s = pool.tile([M_OUT, ROWS_PER_CORE, W], f32)
            wtp = pool.tile([2 * C, 6 * M_OUT], mm_dtype)
            nc.scalar.dma_start(wtp[:], wtp_in[:])

            chunks = []
            for t in range(N_BATCHES):
                ck = pool.tile([2 * C, CHUNK_W], mm_dtype, tag=f"chunk{t}")
                lo = WT_W + t * CHUNK_W
                # rows 0-63: the chunk; rows 64-127: same, shifted one row.
                # Two HWDGE queues (SP / Activation) so both transfer at once.
                # (Splitting further onto SWDGE queues was measured SLOWER —
                # SWDGE first-byte latency exceeds the parallelism gain.)
                nc.sync.dma_start(ck[:C], fused_in[:, lo : lo + CHUNK_W])
                nc.scalar.dma_start(
                    ck[C:, :DUP_W], fused_in[:, lo + WPAD : lo + WPAD + DUP_W]
                )
                chunks.append(ck)

            for t in range(N_BATCHES):
                # flat [12, 512] psum view: fp32r matmuls only hit the
                # 1-cycle/row fast path when the dst innermost run is >=256
                ps = psum.tile([M_OUT, ROW_BATCH * W], f32)
                f3_pair = chunks[t][:].rearrange("c (r w) -> c r w", w=WPAD)
                f3_solo = chunks[t][:C].rearrange("c (r w) -> c r w", w=WPAD)
                # output rows y = 4t + r need feature row u = y + p - 1 =
                # chunk-local row p + r; output col x needs padded col q + x.
                for idx in range(2 * K):
                    q = idx % K
                    if idx < K:  # paired taps (0,q) + (1,q), K = 128
                        lhsT = wtp[:, q * M_OUT : (q + 1) * M_OUT]
                        rhs = f3_pair[:, 0:ROW_BATCH, q : q + W]
                    else:  # solo tap (2,q), K = 64
                        lhsT = wtp[:C, (K + q) * M_OUT : (K + q + 1) * M_OUT]
                        rhs = f3_solo[:, 2 : 2 + ROW_BATCH, q : q + W]
                    nc.tensor.matmul(
                        ps[:],
                        lhsT,
                        rhs,
                        start=(idx == 0),
                        stop=(idx == 2 * K - 1),
                    )
                # (splitting the last copy+DMA into halves was measured
                # ~0.9us slower in matched samples — keep it whole)
                nc.vector.tensor_copy(
                    out_s[:, 4 * t : 4 * t + ROW_BATCH],
                    ps[:].rearrange("m (r w) -> m r w", w=W),
                )
                eng = nc.scalar if t % 2 == 0 else nc.sync
                eng.dma_start(
                    out_d[:, 4 * t : 4 * t + ROW_BATCH],
                    out_s[:, 4 * t : 4 * t + ROW_BATCH],
                )

    nc.finalize()
    return nc


def _get_program(mm_dtype_name="float32r"):
    key = (mm_dtype_name, N_WARM, PSUM_BUFS)
    if key not in _CACHE:
        _CACHE[key] = _build_program(getattr(mybir.dt, mm_dtype_name))
    return _CACHE[key]


def _host_weights(kernel_1, bias_1, kernel_2, bias_2, kernel_3, bias_3):
    """4 parity rows through the MLP -> wt [C, 9, 12] fp32."""
    v4 = np.array(
        [[0.5 * a, 0.5 * b, 0.5] for a in range(2) for b in range(2)],
        dtype=np.float32,
    )
    h = np.maximum(v4 @ kernel_1 + bias_1, 0.0).astype(np.float32)
    h = np.maximum(h @ kernel_2 + bias_2, 0.0).astype(np.float32)
    w = (h @ kernel_3 + bias_3).astype(np.float32)          # [4, 3*3*C*3]
    w = w.reshape(2, 2, K, K, C, OUT_C)                     # [a, b, p, q, c, o]
    wt = np.transpose(w, (4, 2, 3, 0, 1, 5))                # [c, p, q, a, b, o]
    return np.ascontiguousarray(wt.reshape(C, K * K, M_OUT), dtype=np.float32)


def _host_feat_slices(feature):
    """Per-core zero-padded [C, 19, 130] slices of feature^T (one trailing
    zero row so the per-batch 7-row chunks slice uniformly)."""
    featT = np.ascontiguousarray(np.transpose(feature, (2, 0, 1)))  # [C, H, W]
    slices = []
    for k in range(N_CORES):
        fs = np.zeros((C, HALO_ROWS + 1, WPAD), dtype=np.float32)
        y0 = k * ROWS_PER_CORE
        lo = max(y0 - 1, 0)
        hi = min(y0 + ROWS_PER_CORE + 1, H)
        fs[:, (lo - (y0 - 1)) : (hi - (y0 - 1)), 1 : 1 + W] = featT[:, lo:hi, :]
        slices.append(fs)
    return slices


def _host_pack(wt, fslice):
    """wt [C, 9, 12] + one core's [C, 19, 130] slice -> fused [C, FUSED_W]."""
    parts = [wt.reshape(C, WT_W)] + [
        fslice[:, ROW_BATCH * t : ROW_BATCH * t + CHUNK_R].reshape(C, -1)
        for t in range(N_BATCHES)
    ]
    return np.ascontiguousarray(np.concatenate(parts, axis=1))


def _host_pack_wtp(wt):
    """wt [C, 9(p*3+q), 12] -> [2C, 72]: cols 0-35 pair q0..2 (p=0 top /
    p=1 bottom), cols 36-71 solo p=2 (top, zero bottom)."""
    w3 = wt.reshape(C, K, K, M_OUT)                      # [c, p, q, m]
    pairs = np.concatenate(
        [w3[:, 0].reshape(C, K * M_OUT), w3[:, 1].reshape(C, K * M_OUT)], axis=0
    )                                                    # [2C, 36]
    solos = np.concatenate(
        [w3[:, 2].reshape(C, K * M_OUT), np.zeros((C, K * M_OUT), np.float32)],
        axis=0,
    )                                                    # [2C, 36]
    return np.ascontiguousarray(np.concatenate([pairs, solos], axis=1))


def _assemble(results):
    """Per-core [m_rows, 16, 128] outputs -> full [H*W*4, 3]."""
    big = np.empty((SCALE * H, SCALE * W, OUT_C), dtype=np.float32)
    for k in range(N_CORES):
        arr = np.asarray(results[k]["out"])
        if arr.shape[0] > M_OUT:  # col-tiled: sum the two chain slices
            ofs = arr.shape[0] - M_OUT
            arr = arr[0:M_OUT] + arr[ofs : ofs + M_OUT]
        o = arr.reshape(2, 2, OUT_C, ROWS_PER_CORE, W)
        # [a, b, o, y, x] -> [y, a, x, b, o]
        t = np.transpose(o, (3, 0, 4, 1, 2)).reshape(
            SCALE * ROWS_PER_CORE, SCALE * W, OUT_C
        )
        big[SCALE * ROWS_PER_CORE * k : SCALE * ROWS_PER_CORE * (k + 1)] = t
    return big.reshape(-1, OUT_C)


def run_device(inputs, mm_dtype_name="float32r", trace=False, **run_kwargs):
    feature = np.asarray(inputs["feature"], dtype=np.float32)
    scale = int(np.asarray(inputs["scale"]))
    assert scale == SCALE, f"kernel specialized for scale=2, got {scale}"
    assert feature.shape == (H, W, C)

    wt = _host_weights(
        np.asarray(inputs["kernel_1"], np.float32),
        np.asarray(inputs["bias_1"], np.float32),
        np.asarray(inputs["kernel_2"], np.float32),
        np.asarray(inputs["bias_2"], np.float32),
        np.asarray(inputs["kernel_3"], np.float32),
        np.asarray(inputs["bias_3"], np.float32),
    )
    slices = _host_feat_slices(feature)
    wtp = _host_pack_wtp(wt)
    in_maps = [
        {"fused": _host_pack(wt, slices[k]), "wtp": wtp} for k in range(N_CORES)
    ]
    nc = _get_program(mm_dtype_name)
    res = run_bass_kernel_spmd(
        nc, in_maps, list(range(N_CORES)), trace=trace, **run_kwargs
    )
    return _assemble(res.results), res


def kernel(**inputs) -> np.ndarray:
    out, _ = run_device(inputs)
    return out
